# revision 4
# baseline (speedup 1.0000x reference)
"""MDCT kernel for Trainium2 (8 NeuronCores, batch-parallel).

Math: MDCT = TDAC fold + DCT-IV (N = 1024), halving the matmul work vs
the direct 2048x1024 frame matrix:
    out[f, k] = sum_m y[f, m] * D[m, k],   D[m, k] = sqrt(2/N) cos(pi/N (m+.5)(k+.5))
with the folded frame y built from X2 = x.reshape(1024, 1024) rows
(hop = 1024; row f-1 and row f make up frame f, center-padded):
    y[f, 0:512]    = G[f]   (= 0 for f = 1024)
    y[f, 512:1024] = H[f-1] (= 0 for f = 0)
    G[r, j] = -w[1535-j] X2[r, 511-j] - w[1536+j] X2[r, 512+j]
    H[r, j] =  w[j]      X2[r, j]     - w[1023-j] X2[r, 1023-j]

Per-core layout: one batch channel per NeuronCore, all fp16 on chip
(PSUM accumulates fp32).  Per 128-row x tile: DVE computes G/H (reversed
free-dim APs), the PE transposes them into gt/ht[m, frame] (frame axis
offset by one in ht so the f-1 alignment is a free-dim offset), then 16
fp16 matmuls per 128-frame tile accumulate the DCT-IV in PSUM.
"""

import numpy as np

import concourse.bass as bass
import concourse.bacc as bacc
import concourse.mybir as mybir
import concourse.tile as tile
from concourse import masks
from concourse.bass_utils import run_bass_kernel_spmd

B = 8
T = 1 << 20
R = 1024          # rows of X2 per channel (T // hop)
CN = 1024         # row width (hop)
NF = 1025         # output frames
NK = 1024         # output bins
F16 = mybir.dt.float16
F32 = mybir.dt.float32

_NC_CACHE = None
_D_CACHE = None
_WF_CACHE = None


def build_nc() -> bass.Bass:
    nc = bacc.Bacc("TRN2", target_bir_lowering=False, debug=False)
    x = nc.dram_tensor("x", [R, CN], F16, kind="ExternalInput").ap()
    wf = nc.dram_tensor("wf", [1, 4 * 512], F16, kind="ExternalInput").ap()
    d = nc.dram_tensor("d", [CN, NK], F16, kind="ExternalInput").ap()
    out = nc.dram_tensor("out", [NF, NK], F16, kind="ExternalOutput").ap()

    with tile.TileContext(nc) as tc:
        with (
            tc.tile_pool(name="persist", bufs=1) as persist,
            tc.tile_pool(name="xin", bufs=3) as xin,
            tc.tile_pool(name="gh", bufs=3) as gh,
            tc.tile_pool(name="outp", bufs=3) as outp,
            tc.tile_pool(name="tps", bufs=2, space="PSUM") as tps,
            tc.tile_pool(name="mmps", bufs=4, space="PSUM") as mmps,
        ):
            # DCT-IV matrix, 8 row chunks: ds[p, c, k] = d[128c + p, k]
            ds = persist.tile([128, 8, NK], F16)
            d_r = d.rearrange("(c p) k -> p c k", p=128)
            for c in range(8):
                nc.sync.dma_start(ds[:, c, :], d_r[:, c, :])

            # fold window vectors, broadcast to all partitions:
            # wfb[:, 0:4, :] = gA, gB, hA, hB
            wfv = persist.tile([1, 4, 512], F16)
            wfb = persist.tile([128, 4, 512], F16)
            nc.sync.dma_start(wfv[:], wf.rearrange("o (c j) -> o c j", c=4))
            nc.gpsimd.partition_broadcast(wfb[:], wfv[:])

            ident = persist.tile([128, 128], F16)
            masks.make_identity(nc, ident[:])

            # gt[p, c, f] = G[f, 128c + p]  (f < 1024; col 1024 is zero)
            # ht[p, c, 1 + r] = H[r, 128c + p]  (col 0 is zero)
            gt = persist.tile([128, 4, NF], F16)
            ht = persist.tile([128, 4, NF], F16)
            nc.vector.memset(gt[:, :, 1024:1025], 0.0)
            nc.vector.memset(ht[:, :, 0:1], 0.0)

            def load_fold_transpose(i: int):
                r0 = i * 128
                xt = xin.tile([128, CN], F16)
                nc.sync.dma_start(xt[:], x[r0:r0 + 128, :])
                g = gh.tile([128, 512], F16, tag="g")
                h = gh.tile([128, 512], F16, tag="h")
                t1 = gh.tile([128, 512], F16, tag="t1")
                t2 = gh.tile([128, 512], F16, tag="t2")
                mul = mybir.AluOpType.mult
                add = mybir.AluOpType.add
                nc.vector.tensor_tensor(g[:], xt[:, 511::-1], wfb[:, 0, :], op=mul)
                nc.vector.tensor_tensor(t1[:], xt[:, 512:], wfb[:, 1, :], op=mul)
                nc.vector.tensor_tensor(g[:], g[:], t1[:], op=add)
                nc.vector.tensor_tensor(h[:], xt[:, 0:512], wfb[:, 2, :], op=mul)
                nc.vector.tensor_tensor(t2[:], xt[:, :511:-1], wfb[:, 3, :], op=mul)
                nc.vector.tensor_tensor(h[:], h[:], t2[:], op=add)
                for c in range(4):
                    pg = tps.tile([128, 128], F16)
                    nc.tensor.transpose(pg[:], g[:, c * 128:(c + 1) * 128], ident[:])
                    nc.scalar.copy(gt[:, c, r0:r0 + 128], pg[:])
                    ph = tps.tile([128, 128], F16)
                    nc.tensor.transpose(ph[:], h[:, c * 128:(c + 1) * 128], ident[:])
                    nc.scalar.copy(ht[:, c, 1 + r0:1 + r0 + 128], ph[:])

            def dct4_tile(j: int):
                f0 = j * 128
                pa = mmps.tile([128, 512], F32, tag="mm")
                pb = mmps.tile([128, 512], F32, tag="mm")
                for c in range(8):
                    if c < 4:
                        w = gt[:, c, f0:f0 + 128]
                    else:
                        w = ht[:, c - 4, f0:f0 + 128]
                    nc.tensor.matmul(
                        pa[:], w, ds[:, c, 0:512],
                        start=(c == 0), stop=(c == 7),
                    )
                    nc.tensor.matmul(
                        pb[:], w, ds[:, c, 512:1024],
                        start=(c == 0), stop=(c == 7),
                    )
                ot = outp.tile([128, NK], F16)
                nc.vector.tensor_copy(ot[:, 0:512], pa[:])
                nc.vector.tensor_copy(ot[:, 512:1024], pb[:])
                nc.sync.dma_start(out[f0:f0 + 128, :], ot[:])

            load_fold_transpose(0)
            for j in range(8):
                if j < 7:
                    load_fold_transpose(j + 1)
                dct4_tile(j)

            # Last frame (f = 1024): first half of y is zero, so only ht.
            pa = mmps.tile([1, 512], F32, tag="mm")
            pb = mmps.tile([1, 512], F32, tag="mm")
            for c in range(4, 8):
                w = ht[:, c - 4, 1024:1025]
                nc.tensor.matmul(
                    pa[:], w, ds[:, c, 0:512],
                    start=(c == 4), stop=(c == 7),
                )
                nc.tensor.matmul(
                    pb[:], w, ds[:, c, 512:1024],
                    start=(c == 4), stop=(c == 7),
                )
            ot = outp.tile([1, NK], F16, tag="ot_last")
            nc.vector.tensor_copy(ot[:, 0:512], pa[:])
            nc.vector.tensor_copy(ot[:, 512:1024], pb[:])
            nc.sync.dma_start(out[1024:1025, :], ot[:])

    return nc


def make_d() -> np.ndarray:
    m = np.arange(CN, dtype=np.float64)[:, None]
    k = np.arange(NK, dtype=np.float64)[None, :]
    d = np.sqrt(2.0 / NK) * np.cos(np.pi / NK * (m + 0.5) * (k + 0.5))
    return d.astype(np.float16)


def make_wf(window: np.ndarray) -> np.ndarray:
    w = window.astype(np.float64)
    j = np.arange(512)
    gA = -w[1535 - j]
    gB = -w[1536 + j]
    hA = w[j]
    hB = -w[1023 - j]
    return np.concatenate([gA, gB, hA, hB]).astype(np.float16)[None, :]


def _get_nc() -> bass.Bass:
    global _NC_CACHE
    if _NC_CACHE is None:
        _NC_CACHE = build_nc()
        _NC_CACHE.compile()
    return _NC_CACHE


def run_spmd(x: np.ndarray, window: np.ndarray, **kwargs):
    """Shard, run on 8 cores, return (stacked output, BassKernelResults)."""
    global _D_CACHE, _WF_CACHE
    if _D_CACHE is None:
        _D_CACHE = make_d()
    if _WF_CACHE is None or _WF_CACHE[0] != window.tobytes():
        _WF_CACHE = (window.tobytes(), make_wf(window))
    wf = _WF_CACHE[1]
    x16 = x.astype(np.float16).reshape(B, R, CN)
    in_maps = [
        {"x": np.ascontiguousarray(x16[b]), "wf": wf, "d": _D_CACHE}
        for b in range(B)
    ]
    res = run_bass_kernel_spmd(nc=_get_nc(), in_maps=in_maps,
                               core_ids=list(range(B)), **kwargs)
    out = np.stack(
        [res.results[b]["out"].astype(np.float32) for b in range(B)], axis=0
    )
    return out, res


def kernel(x: np.ndarray, window: np.ndarray) -> np.ndarray:
    out, _ = run_spmd(np.asarray(x), np.asarray(window))
    return out
